# revision 1
# baseline (speedup 1.0000x reference)
"""Trainium2 Bass kernel for nn_KTopPooling (8-core SPMD).

Algorithm (per core, one SPMD program, per-core variability passed as data):
  Host shards nodes across 8 cores on graph boundaries (batch is sorted).
  Phase 1 (memory-bound stream): xT [256, NC_CAP] (host-pretransposed) is
    streamed in 1MB tiles; h^T = leaky(W1^T xT + b1) via fp32 matmuls with two
    512-node subchunks packed on PSUM partitions (block-diagonal W2 computes
    both subchunks' scores in one matmul); scores^T [3, n] stored to DRAM.
  Phase 2: per-graph segments regrouped into a dense [GCAP, 3, L] layout via
    overlapping fixed-length indirect-DMA gathers (offsets = host-computed
    segment starts); an additive -1e30 mask kills padding slots. Segment
    max / argmax via vector.max + max_index, softmax denominator via
    Exp activation with accum_out. sg = exp(m) / sum(exp(s)).
  Phase 3: winning x rows gathered from the natural-layout shard by
    device-computed indices (indirect DMA), scaled by sg, PE-transposed, and
    pushed through the head matmul + leaky. Host concatenates per-core
    [GCAP, 256] outputs.
"""
import os
import numpy as np

import concourse.bass as bass
import concourse.bacc as bacc
import concourse.tile as tile
from concourse import mybir
from concourse.bass_utils import run_bass_kernel_spmd

f32 = mybir.dt.float32
i32 = mybir.dt.int32
u32 = mybir.dt.uint32
AF = mybir.ActivationFunctionType
ALU = mybir.AluOpType

# problem constants (hardcoded per harness contract)
N, C, H, K, G = 200000, 256, 64, 3, 512
NCORES = 8
ALPHA = 0.01
NEG = -1.0e30


class Cfg:
    def __init__(self, nc_cap=25600, gcap=80, L=512, dma_t=2048):
        assert nc_cap % 1024 == 0 and dma_t % 1024 == 0
        self.nc_cap = nc_cap
        self.gcap = gcap
        self.L = L
        self.dma_t = dma_t
        self.ncs = nc_cap + L  # scoresT row length (OOB slack for gathers)


def build(cfg: Cfg):
    nc = bacc.Bacc("TRN2", target_bir_lowering=False, debug=False,
                   num_devices=NCORES)

    NC_CAP, GCAP, L, NCS = cfg.nc_cap, cfg.gcap, cfg.L, cfg.ncs

    xT_d = nc.dram_tensor("xT", [C, NC_CAP], f32, kind="ExternalInput")
    xrows_d = nc.dram_tensor("xrows", [NC_CAP, C], f32, kind="ExternalInput")
    w1_d = nc.dram_tensor("w1", [C, H], f32, kind="ExternalInput")
    b1bd_d = nc.dram_tensor("b1bd", [128, 1], f32, kind="ExternalInput")
    w2bd_d = nc.dram_tensor("w2bd", [128, 2 * K], f32, kind="ExternalInput")
    b2bd_d = nc.dram_tensor("b2bd", [2 * K, 1], f32, kind="ExternalInput")
    wh_d = nc.dram_tensor("wh", [K * C, C], f32, kind="ExternalInput")
    bhr_d = nc.dram_tensor("bhr", [128, C], f32, kind="ExternalInput")
    iden_d = nc.dram_tensor("iden", [128, 128], f32, kind="ExternalInput")
    mask_d = nc.dram_tensor("mask", [GCAP, K, L], f32, kind="ExternalInput")
    segi_d = nc.dram_tensor("segi", [GCAP, 1], i32, kind="ExternalInput")
    segf_d = nc.dram_tensor("segf", [GCAP, 1], f32, kind="ExternalInput")

    out_d = nc.dram_tensor("out", [GCAP, C], f32, kind="ExternalOutput")

    with tile.TileContext(nc) as tc:
        import contextlib
        with contextlib.ExitStack() as ctx:
            s1 = ctx.enter_context(tc.tile_pool(name="singles", bufs=1))
            lp = ctx.enter_context(tc.tile_pool(name="loads", bufs=3))
            hp = ctx.enter_context(tc.tile_pool(name="hbuf", bufs=3))
            sp = ctx.enter_context(tc.tile_pool(name="sbuf_s", bufs=3))
            pph = ctx.enter_context(tc.tile_pool(name="ph", bufs=2, space="PSUM"))
            pps = ctx.enter_context(tc.tile_pool(name="ps", bufs=2, space="PSUM"))
            ppo = ctx.enter_context(tc.tile_pool(name="po", bufs=2, space="PSUM"))
            dp = ctx.enter_context(tc.tile_pool(name="dram", bufs=1, space="DRAM"))

            scoresT = dp.tile([K, NCS], f32)

            # ---- constants / small inputs ----
            w1sb = s1.tile([128, 2, H], f32)
            nc.sync.dma_start(out=w1sb[:],
                              in_=w1_d[:].rearrange("(ch p) m -> p ch m", p=128))
            b1bd = s1.tile([128, 1], f32)
            nc.sync.dma_start(out=b1bd[:], in_=b1bd_d[:])
            w2bd = s1.tile([128, 2 * K], f32)
            nc.sync.dma_start(out=w2bd[:], in_=w2bd_d[:])
            b2bd = s1.tile([2 * K, 1], f32)
            nc.sync.dma_start(out=b2bd[:], in_=b2bd_d[:])
            whsb = s1.tile([128, 2 * K, C], f32)
            nc.sync.dma_start(out=whsb[:],
                              in_=wh_d[:].rearrange("(blk p) c -> p blk c", p=128))
            bhr = s1.tile([128, C], f32)
            nc.sync.dma_start(out=bhr[:], in_=bhr_d[:])
            iden = s1.tile([128, 128], f32)
            nc.sync.dma_start(out=iden[:], in_=iden_d[:])
            msk = s1.tile([GCAP, K, L], f32)
            nc.sync.dma_start(out=msk[:], in_=mask_d[:])
            segi = s1.tile([GCAP, 1], i32)
            nc.sync.dma_start(out=segi[:], in_=segi_d[:])
            segf = s1.tile([GCAP, 1], f32)
            nc.sync.dma_start(out=segf[:], in_=segf_d[:])

            # zero the gather-slack tail of scoresT
            ztile = s1.tile([K, L], f32)
            nc.vector.memset(ztile[:], 0.0)
            nc.sync.dma_start(out=scoresT[:, NC_CAP:], in_=ztile[:])

            # ---- phase 1: stream x, compute scores^T ----
            for n0 in range(0, NC_CAP, cfg.dma_t):
                nt = min(cfg.dma_t, NC_CAP - n0)
                xt = lp.tile([128, 2, cfg.dma_t], f32, tag="xt")
                nc.sync.dma_start(
                    out=xt[:, :, :nt],
                    in_=xT_d[:].rearrange("(ch p) n -> p ch n", p=128)[:, :, n0:n0 + nt],
                )
                for s0 in range(0, nt, 1024):
                    ph = pph.tile([128, 512], f32, tag="ph")
                    for half in (0, 1):
                        for ch in (0, 1):
                            nc.tensor.matmul(
                                out=ph[half * H:(half + 1) * H, :],
                                lhsT=w1sb[:, ch, :],
                                rhs=xt[:, ch, s0 + half * 512: s0 + half * 512 + 512],
                                start=(ch == 0),
                                stop=(ch == 1),
                            )
                    hsb = hp.tile([128, 512], f32, tag="h")
                    nc.scalar.activation(out=hsb[:], in_=ph[:], func=AF.Lrelu,
                                         bias=b1bd[:], alpha=ALPHA)
                    ps = pps.tile([2 * K, 512], f32, tag="ps")
                    nc.tensor.matmul(out=ps[:], lhsT=w2bd[:], rhs=hsb[:],
                                     start=True, stop=True)
                    ssb = sp.tile([2 * K, 512], f32, tag="ssb")
                    nc.scalar.activation(out=ssb[:], in_=ps[:], func=AF.Identity,
                                         bias=b2bd[:])
                    gn0 = n0 + s0
                    nc.sync.dma_start(out=scoresT[:, gn0:gn0 + 512],
                                      in_=ssb[0:K, :])
                    nc.sync.dma_start(out=scoresT[:, gn0 + 512:gn0 + 1024],
                                      in_=ssb[K:2 * K, :])

            # ---- phase 2: segment regroup + softmax stats + argmax ----
            scat = s1.tile([GCAP, K, L], f32)
            for k in range(K):
                nc.gpsimd.indirect_dma_start(
                    out=scat[:, k, :],
                    out_offset=None,
                    in_=scoresT[:],
                    in_offset=bass.IndirectOffsetOnAxis(ap=segi[:], axis=1),
                    element_offset=k * NCS,
                )
            smask = s1.tile([GCAP, K, L], f32)
            nc.vector.tensor_tensor(out=smask[:], in0=scat[:], in1=msk[:],
                                    op=ALU.add)

            den = s1.tile([GCAP, K], f32)
            m1 = s1.tile([GCAP, K], f32)
            idxf = s1.tile([GCAP, K], f32)
            ep = ctx.enter_context(tc.tile_pool(name="expse", bufs=2))
            mp = ctx.enter_context(tc.tile_pool(name="m8p", bufs=2))
            for k in range(K):
                m8 = mp.tile([GCAP, 8], f32, tag="m8")
                nc.vector.max(out=m8[:], in_=smask[:, k, :])
                i8 = mp.tile([GCAP, 8], u32, tag="i8")
                nc.vector.max_index(out=i8[:], in_max=m8[:], in_values=smask[:, k, :])
                nc.vector.tensor_copy(out=m1[:, k:k + 1], in_=m8[:, 0:1])
                nc.vector.tensor_copy(out=idxf[:, k:k + 1], in_=i8[:, 0:1])
                e = ep.tile([GCAP, L], f32, tag="e")
                nc.scalar.activation(out=e[:], in_=smask[:, k, :], func=AF.Exp,
                                     accum_out=den[:, k:k + 1])

            expm = s1.tile([GCAP, K], f32)
            nc.scalar.activation(out=expm[:], in_=m1[:], func=AF.Exp)
            rec = s1.tile([GCAP, K], f32)
            nc.vector.reciprocal(out=rec[:], in_=den[:])
            sg = s1.tile([GCAP, K], f32)
            nc.vector.tensor_tensor(out=sg[:], in0=expm[:], in1=rec[:], op=ALU.mult)

            idxn = s1.tile([GCAP, K], f32)
            nc.vector.tensor_scalar(out=idxn[:], in0=idxf[:], scalar1=segf[:],
                                    scalar2=None, op0=ALU.add)
            idxi = s1.tile([GCAP, K], i32)
            nc.vector.tensor_copy(out=idxi[:], in_=idxn[:])

            # ---- phase 3: gather winners, scale, transpose, head MLP ----
            fT = s1.tile([128, 2 * K, GCAP], f32)
            gp = ctx.enter_context(tc.tile_pool(name="gather", bufs=2))
            ppt = ctx.enter_context(tc.tile_pool(name="pt", bufs=2, space="PSUM"))
            for k in range(K):
                xg = gp.tile([GCAP, C], f32, tag="xg")
                nc.gpsimd.indirect_dma_start(
                    out=xg[:],
                    out_offset=None,
                    in_=xrows_d[:],
                    in_offset=bass.IndirectOffsetOnAxis(ap=idxi[:, k:k + 1], axis=0),
                )
                xgs = gp.tile([GCAP, C], f32, tag="xgs")
                nc.vector.tensor_scalar(out=xgs[:], in0=xg[:],
                                        scalar1=sg[:, k:k + 1], scalar2=None,
                                        op0=ALU.mult)
                for ch in (0, 1):
                    pt = ppt.tile([128, GCAP], f32, tag="pt")
                    nc.tensor.transpose(out=pt[:], in_=xgs[:, ch * 128:(ch + 1) * 128],
                                        identity=iden[0:GCAP, 0:GCAP])
                    nc.vector.tensor_copy(out=fT[:, k * 2 + ch, :], in_=pt[:])

            po = ppo.tile([GCAP, C], f32)
            for blk in range(2 * K):
                nc.tensor.matmul(out=po[:], lhsT=fT[:, blk, :], rhs=whsb[:, blk, :],
                                 start=(blk == 0), stop=(blk == 2 * K - 1))
            ob1 = s1.tile([GCAP, C], f32)
            nc.vector.tensor_tensor(out=ob1[:], in0=po[:], in1=bhr[0:GCAP, :],
                                    op=ALU.add)
            ob2 = s1.tile([GCAP, C], f32)
            nc.scalar.activation(out=ob2[:], in_=ob1[:], func=AF.Lrelu, alpha=ALPHA)
            nc.sync.dma_start(out=out_d[:], in_=ob2[:])

    nc.compile()
    return nc


def shard(batch, cfg: Cfg):
    """Partition graphs across cores on graph boundaries, balanced by nodes."""
    counts = np.bincount(batch.astype(np.int64), minlength=G)
    cum = np.zeros(G + 1, dtype=np.int64)
    cum[1:] = np.cumsum(counts)
    ntot = int(cum[-1])
    gsplit = [0]
    for i in range(1, NCORES):
        target = ntot * i // NCORES
        s = int(np.searchsorted(cum, target))
        # pick the boundary closest to target
        if s > 0 and abs(int(cum[s - 1]) - target) < abs(int(cum[s]) - target):
            s -= 1
        s = max(gsplit[-1], min(s, G))
        gsplit.append(s)
    gsplit.append(G)
    return counts, cum, gsplit


_BUILD_CACHE = {}


def _get_nc(cfg: Cfg):
    key = (cfg.nc_cap, cfg.gcap, cfg.L, cfg.dma_t)
    if key not in _BUILD_CACHE:
        _BUILD_CACHE[key] = build(cfg)
    return _BUILD_CACHE[key]


def make_in_maps(x, batch, W1, b1, W2, b2, Wh, bh, cfg: Cfg):
    NC_CAP, GCAP, L = cfg.nc_cap, cfg.gcap, cfg.L
    counts, cum, gsplit = shard(batch, cfg)
    assert counts.min() > 0, "empty graph unsupported"

    w1 = np.ascontiguousarray(W1, dtype=np.float32)
    b1bd = np.concatenate([b1, b1]).astype(np.float32).reshape(128, 1)
    w2bd = np.zeros((128, 2 * K), dtype=np.float32)
    w2bd[0:H, 0:K] = W2
    w2bd[H:2 * H, K:2 * K] = W2
    b2bd = np.concatenate([b2, b2]).astype(np.float32).reshape(2 * K, 1)
    wh = np.ascontiguousarray(Wh, dtype=np.float32)
    bhr = np.broadcast_to(bh.astype(np.float32), (128, C)).copy()
    iden = np.eye(128, dtype=np.float32)

    xTfull = np.ascontiguousarray(x.T, dtype=np.float32)  # [C, N]

    in_maps = []
    meta = []
    for ci in range(NCORES):
        g0, g1 = gsplit[ci], gsplit[ci + 1]
        n0, n1 = int(cum[g0]), int(cum[g1])
        ncn = n1 - n0
        gcn = g1 - g0
        assert ncn <= NC_CAP, f"core {ci}: {ncn} nodes > cap {NC_CAP}"
        assert gcn <= GCAP, f"core {ci}: {gcn} graphs > cap {GCAP}"

        xT = np.zeros((C, NC_CAP), dtype=np.float32)
        xT[:, :ncn] = xTfull[:, n0:n1]
        xrows = np.zeros((NC_CAP, C), dtype=np.float32)
        xrows[:ncn] = x[n0:n1]

        seg = np.zeros((GCAP,), dtype=np.int64)
        seg[:gcn] = cum[g0:g1] - n0
        lens = np.zeros((GCAP,), dtype=np.int64)
        lens[:gcn] = counts[g0:g1]

        # additive mask: 0 for valid slots, NEG for padding slots of real
        # graphs; all-zero rows for padded graphs (avoids NaN; host discards).
        mask = np.zeros((GCAP, K, L), dtype=np.float32)
        slot = np.arange(L)[None, :] >= lens[:gcn, None]  # [gcn, L]
        mask[:gcn, :, :] = np.where(slot[:, None, :], NEG, 0.0)

        in_maps.append({
            "xT": xT,
            "xrows": xrows,
            "w1": w1,
            "b1bd": b1bd,
            "w2bd": w2bd,
            "b2bd": b2bd,
            "wh": wh,
            "bhr": bhr,
            "iden": iden,
            "mask": mask,
            "segi": seg.astype(np.int32).reshape(GCAP, 1),
            "segf": seg.astype(np.float32).reshape(GCAP, 1),
        })
        meta.append((g0, g1))
    return in_maps, meta


def _run(inputs, cfg=None, trace=False):
    cfg = cfg or Cfg()
    x = np.asarray(inputs["x"], dtype=np.float32)
    batch = np.asarray(inputs["batch"])
    W1 = np.asarray(inputs["W1"], dtype=np.float32)
    b1 = np.asarray(inputs["b1"], dtype=np.float32)
    W2 = np.asarray(inputs["W2"], dtype=np.float32)
    b2 = np.asarray(inputs["b2"], dtype=np.float32)
    Wh = np.asarray(inputs["Wh"], dtype=np.float32)
    bh = np.asarray(inputs["bh"], dtype=np.float32)

    in_maps, meta = make_in_maps(x, batch, W1, b1, W2, b2, Wh, bh, cfg)
    nc = _get_nc(cfg)
    res = run_bass_kernel_spmd(nc, in_maps, core_ids=list(range(NCORES)),
                               trace=trace)
    out = np.zeros((G, C), dtype=np.float32)
    for ci, (g0, g1) in enumerate(meta):
        out[g0:g1] = res.results[ci]["out"][:g1 - g0]
    return out, res


def kernel(**inputs):
    out, _ = _run(inputs)
    return out
